# revision 10
# baseline (speedup 1.0000x reference)
"""CLIP text transformer with prompt tuning on 8 TRN2 NeuronCores — v3.

Data-parallel over batch (16 seqs/core). v3 restructures for PE density:
- attention chains run inside the QKV GEMM phase (per-chunk wavefront), so
  the scalar engine's program order is [ln/exp ...][gelu ...] per layer:
  2 activation-table loads per layer instead of ~18.
- LN stats are software-pipelined: LN1 stats of layer i+1 are emitted right
  after each W2 tile of layer i; LN2 stats right after each Wo tile. The
  ~3us serial LN chains hide under GEMMs instead of stalling the PE.
- quadrant (64-partition) score matmuls: no zero-split K copies, head pairs
  run concurrently on distinct PE row groups.
- each weight matrix gets its own tile-pool tag so DMAs don't serialize
  behind ring-slot reuse.
- causal mask multiply + LN apply moved to the (idle) GPSIMD engine.
"""
import sys

sys.path.insert(0, "/opt/trn_rl_repo")

import json

import ml_dtypes
import numpy as np

import concourse.bass as bass
import concourse.mybir as mybir
import concourse.tile as tile

# ----------------------------------------------------------------------------
# walrus in this container rejects instructions with more than one sync wait /
# update; split them into chains of single-wait Drains before compiling.
import concourse.bass2jax as _b2j

_orig_compile_bir = _b2j.compile_bir_kernel


def _split_sync(bir_bytes):
    bir = json.loads(bir_bytes)
    for fn in bir.get("functions", []):
        for bb in fn.get("blocks", []):
            new = []
            for inst in bb.get("instructions", []):
                si = inst.get("sync_info")
                waits = (si or {}).get("on_wait") or []
                if len(waits) > 1:
                    for i, w in enumerate(waits[:-1]):
                        new.append({
                            "debug": inst.get("debug", 0),
                            "engine": inst["engine"],
                            "ins": [], "outs": [],
                            "name": f"{inst['name']}__w{i}",
                            "opcode": "NoOp",
                            "sync_info": {"on_update": [], "on_wait": [w]},
                        })
                    si["on_wait"] = waits[-1:]
                new.append(inst)
                updates = (si or {}).get("on_update") or []
                if len(updates) > 1:
                    for i, u in enumerate(updates[1:]):
                        new.append({
                            "debug": inst.get("debug", 0),
                            "engine": inst["engine"],
                            "ins": [], "outs": [],
                            "name": f"{inst['name']}__u{i}",
                            "opcode": "NoOp",
                            "sync_info": {"on_update": [u], "on_wait": []},
                        })
                    si["on_update"] = updates[:1]
            bb["instructions"] = new
    return json.dumps(bir).encode()


def _patched_compile_bir(bir_json, tmpdir, neff_name="file.neff"):
    return _orig_compile_bir(_split_sync(bir_json), tmpdir, neff_name)


_b2j.compile_bir_kernel = _patched_compile_bir
# ----------------------------------------------------------------------------

from concourse.bass_utils import run_bass_kernel_spmd

f32 = mybir.dt.float32
bf16 = mybir.dt.bfloat16
AF = mybir.ActivationFunctionType
ALU = mybir.AluOpType

V, P, D, H, NL, FF = 49408, 77, 512, 8, 12, 2048
B, T, NP = 128, 69, 8
DG, DS = 6, 6
EPS = 1e-5
SCALE = 0.125
NCORES = 8
S = T + NP            # 77
BSH = B // NCORES     # 16 sequences per core
NT = BSH * S          # 1232 tokens per core
NTILES = 10           # 1280 padded
DB = D // 128         # 4 feature blocks
FB = FF // 128        # 16 ff blocks
DH = D // H           # 64
CHUNKS = [(0, 512), (512, 512), (1024, 256)]

# engine assignment switches
APPLY_ON_POOL = False      # LN (x-mean)*rsqrt apply on gpsimd
CAUSAL_ON_POOL = False     # pt *= causal on gpsimd
QUADRANT_SCORES = False    # 64-partition score matmuls (no k zero-split)

# last sequence needed before token tile t is fully attention-complete
SEQ_HI = [min(BSH - 1, (128 * (t + 1) - 1) // S) for t in range(NTILES)]
# wo tiles that become ready right after attention(b)
WO_AT = {b: [t for t in range(NTILES) if SEQ_HI[t] == b] for b in range(BSH)}
# P1 wavefront: (ln1 tiles, qk chunk index, seqs newly covered)
GROUPS = [([0, 1, 2, 3], 0, range(0, 6)),
          ([4, 5, 6, 7], 1, range(6, 13)),
          ([8, 9], 2, range(13, 16))]
# LN1 stat groups (tile -> (group idx, idx within group))
LN1_GROUPS = [[0, 1, 2, 3], [4, 5, 6, 7], [8, 9]]
LN1_POS = {t: (g, i) for g, tiles in enumerate(LN1_GROUPS)
           for i, t in enumerate(tiles)}
# LN2 rsqrt batches: emitted right after attention(b)
LN2_BATCH_AT = {6: [0, 1, 2, 3], 9: [4, 5], 13: [6, 7], 15: [8, 9]}
# prompt rows by tile: seq b occupies x_t rows [p0, p0+NP) of tile tl
PROMPT_ROWS = {}
for _b in range(BSH):
    _t0 = _b * S + 1
    PROMPT_ROWS.setdefault(_t0 // 128, []).append((_b, _t0 % 128))


def _bf(x):
    return np.ascontiguousarray(x.astype(ml_dtypes.bfloat16))


def _f32(x):
    return np.ascontiguousarray(x.astype(np.float32))


def build_program():
    nc = bass.Bass()
    dp = nc.declare_dram_parameter
    X0 = dp("x0", [128, NTILES, D], f32, isOutput=False)
    PR = dp("prompts", [NL - 1, BSH, NP, D], f32, isOutput=False)
    WQ = dp("wq", [NL, D, D], bf16, isOutput=False)
    WK = dp("wk", [NL, D, D], bf16, isOutput=False)
    WV = dp("wv", [NL, D, D], bf16, isOutput=False)
    WO = dp("wo", [NL, D, D], bf16, isOutput=False)
    W1 = dp("w1", [NL, D, FF], bf16, isOutput=False)
    W2 = dp("w2", [NL, FF, D], bf16, isOutput=False)
    BQ = dp("bq", [NL, 128, DB], f32, isOutput=False)
    BK = dp("bk", [NL, 128, DB], f32, isOutput=False)
    B1 = dp("b1", [NL, 128, FB], f32, isOutput=False)
    BROW = dp("brow", [NL, 2, D], bf16, isOutput=False)   # bo_eff, b2
    CA = dp("causal", [P, H, 80], bf16, isOutput=False)   # mult mask [k,h,q]
    ID = dp("ident", [128, 128], bf16, isOutput=False)
    OUT = dp("out", [128, NTILES, D], f32, isOutput=True)

    from contextlib import ExitStack
    with tile.TileContext(nc) as tc, ExitStack() as ctx:
        consts = ctx.enter_context(tc.tile_pool(name="consts", bufs=1))
        persist = ctx.enter_context(tc.tile_pool(name="persist", bufs=1))
        wqkv = ctx.enter_context(tc.tile_pool(name="wqkv", bufs=1))
        wmlp = ctx.enter_context(tc.tile_pool(name="wmlp", bufs=1))
        bp = ctx.enter_context(tc.tile_pool(name="bp", bufs=2))
        browp = ctx.enter_context(tc.tile_pool(name="browp", bufs=2))
        hp = ctx.enter_context(tc.tile_pool(name="hp", bufs=4))
        hfmp = ctx.enter_context(tc.tile_pool(name="hfmp", bufs=2))
        mlpp = ctx.enter_context(tc.tile_pool(name="mlpp", bufs=2))
        ptp = ctx.enter_context(tc.tile_pool(name="ptp", bufs=3))
        osbp = ctx.enter_context(tc.tile_pool(name="osbp", bufs=2))
        recp = ctx.enter_context(tc.tile_pool(name="recp", bufs=8))
        statp = ctx.enter_context(tc.tile_pool(name="statp", bufs=8))
        pmm = ctx.enter_context(tc.tile_pool(name="pmm", bufs=4, space="PSUM"))
        psc = ctx.enter_context(tc.tile_pool(name="psc", bufs=2, space="PSUM"))

        ident = consts.tile([128, 128], bf16, tag="ident")
        nc.sync.dma_start(out=ident, in_=ID[:, :])
        causal = consts.tile([P, H, 80], bf16, tag="causal")
        nc.sync.dma_start(out=causal, in_=CA[:, :, :])
        ones_row = consts.tile([1, 128], bf16, tag="ones")
        nc.gpsimd.memset(ones_row, 1.0)
        eps_t = consts.tile([128, 1], f32, tag="eps")
        nc.gpsimd.memset(eps_t, EPS)

        x_t = persist.tile([128, NTILES, D], f32, tag="x")
        nc.sync.dma_start(out=x_t, in_=X0[:, :, :])
        v_tm = persist.tile([128, BSH, H, DH + 1], bf16, tag="v")
        nc.gpsimd.memset(v_tm[0:P, :, :, DH:DH + 1], 1.0)
        q_fm = persist.tile([128, DB, NTILES * 128], bf16, tag="qfm")
        if QUADRANT_SCORES:
            k_fm = persist.tile([128, DB, NTILES * 128], bf16, tag="kfm")
        else:
            k_fm = None
        o_fm = persist.tile([128, DB, NTILES * 128], bf16, tag="ofm")
        nc.gpsimd.memset(o_fm[:, :, NT:], 0.0)

        if not QUADRANT_SCORES:
            k_z0 = persist.tile([128, DB, NTILES * 128], bf16, tag="kz0")
            k_z1 = persist.tile([128, DB, NTILES * 128], bf16, tag="kz1")
            nc.gpsimd.memset(k_z0[64:128, :, :], 0.0)
            nc.gpsimd.memset(k_z1[0:64, :, :], 0.0)

        # ---------- helpers ------------------------------------------------
        def emit_stat(mvb, idx, t):
            st = statp.tile([128, 6], f32, tag="bnst", name="st")
            nc.vector.bn_stats(out=st, in_=x_t[:, t, :])
            nc.vector.bn_aggr(out=mvb[:, idx, :], in_=st)

        def emit_rsqrt(mvb, n):
            lv = statp.tile([128, 4], f32, tag="lv", name="lv")
            nc.scalar.activation(
                out=lv[:, 0:n], in_=mvb[:, 0:n, 1], func=AF.Ln, bias=eps_t)
            rs = statp.tile([128, 4], f32, tag="rs", name="rs")
            nc.scalar.activation(out=rs[:, 0:n], in_=lv[:, 0:n],
                                 func=AF.Exp, scale=-0.5)
            return rs

        def ln_apply_tile(t, mvb, rs, idx, hfm):
            h = hp.tile([128, D], bf16, tag="hln", name="h")
            eng = nc.gpsimd if APPLY_ON_POOL else nc.vector
            eng.tensor_scalar(
                out=h, in0=x_t[:, t, :],
                scalar1=mvb[:, idx, 0:1], scalar2=rs[:, idx:idx + 1],
                op0=ALU.subtract, op1=ALU.mult,
            )
            ps = pmm.tile([128, 512], bf16, tag="pmm", name="ps")
            pv = ps.rearrange("p (db c) -> p db c", db=DB)
            for db in range(DB):
                nc.tensor.transpose(
                    pv[:, db, :], h[:, db * 128:(db + 1) * 128], ident
                )
            nc.scalar.activation(
                out=hfm[:, :, t * 128:(t + 1) * 128], in_=pv, func=AF.Identity
            )

        def qk_chunk(ci, hfm, wq_t, wk_t, bq_t, bk_t):
            c0, cw = CHUNKS[ci]
            for which, w_t, b_t, dst in (("q", wq_t, bq_t, q_fm),
                                         ("k", wk_t, bk_t, k_fm)):
                for m in range(DB):
                    ps = pmm.tile([128, 512], f32, tag="pmm", name="ps")
                    for k in range(DB):
                        nc.tensor.matmul(
                            ps[:, 0:cw],
                            w_t[:, k, m * 128:(m + 1) * 128],
                            hfm[:, k, c0:c0 + cw],
                            start=(k == 0), stop=(k == DB - 1),
                        )
                    if which == "q" or QUADRANT_SCORES:
                        nc.scalar.activation(
                            out=dst[:, m, c0:c0 + cw], in_=ps[:, 0:cw],
                            func=AF.Identity, bias=b_t[:, m:m + 1],
                        )
                    else:
                        nc.vector.tensor_scalar_add(
                            out=k_z0[0:64, m, c0:c0 + cw], in0=ps[0:64, 0:cw],
                            scalar1=b_t[0:64, m:m + 1],
                        )
                        nc.vector.tensor_scalar_add(
                            out=k_z1[64:128, m, c0:c0 + cw], in0=ps[64:128, 0:cw],
                            scalar1=b_t[64:128, m:m + 1],
                        )

        def v_proj(b, hfm, wv_t):
            ps = pmm.tile([128, 512], f32, tag="pmm", name="ps")
            for k in range(DB):
                nc.tensor.matmul(
                    ps[0:P, :],
                    hfm[:, k, b * S:b * S + S],
                    wv_t[:, k, :],
                    start=(k == 0), stop=(k == DB - 1),
                )
            nc.vector.tensor_copy(
                out=v_tm[0:P, b, :, 0:DH],
                in_=ps[0:P, :].rearrange("p (h d) -> p h d", h=H),
            )

        def attention(b):
            bs = b * S
            scf = psc.tile([128, 1024], f32, tag="psc", name="sc")
            sc = scf.rearrange("p (h c) -> p h c", c=128)
            for h in range(H):
                dbl = h // 2
                if QUADRANT_SCORES:
                    p0 = 64 * (h % 2)
                    nc.tensor.matmul(
                        sc[0:P, h, 0:P],
                        k_fm[p0:p0 + 64, dbl, bs:bs + S],
                        q_fm[p0:p0 + 64, dbl, bs:bs + S],
                        start=True, stop=True,
                    )
                else:
                    kz = k_z0 if h % 2 == 0 else k_z1
                    nc.tensor.matmul(
                        sc[0:P, h, 0:P],
                        kz[:, dbl, bs:bs + S],
                        q_fm[:, dbl, bs:bs + S],
                        start=True, stop=True,
                    )
            pt = ptp.tile([128, H, 80], bf16, tag="pt", name="pt")
            nc.scalar.activation(
                out=pt[0:P, :, 0:P], in_=sc[0:P, :, 0:P], func=AF.Exp
            )
            eng = nc.gpsimd if CAUSAL_ON_POOL else nc.vector
            eng.tensor_mul(pt[0:P, :, :], pt[0:P, :, :], causal)
            osb = osbp.tile([128, D], bf16, tag="osb", name="osb")
            for g in range(2):
                ot = pmm.tile([128, 4, DH + 1], f32, tag="pmm", name="ot")
                for hh in range(4):
                    h = g * 4 + hh
                    nc.tensor.matmul(
                        ot[0:P, hh, :],
                        pt[0:P, h, 0:P],
                        v_tm[0:P, b, h, :],
                        start=True, stop=True,
                    )
                rec = recp.tile([128, 4], f32, tag="rec", name="rec")
                nc.vector.reciprocal(
                    out=rec[0:P, :].unsqueeze(2), in_=ot[0:P, :, DH:DH + 1]
                )
                nc.vector.tensor_mul(
                    osb[0:P, g * 256:(g + 1) * 256].rearrange(
                        "p (h d) -> p h d", h=4),
                    ot[0:P, :, 0:DH],
                    rec[0:P, :].unsqueeze(2).broadcast_to((P, 4, DH)),
                )
            po = pmm.tile([128, 512], bf16, tag="pmm", name="po")
            pov = po.rearrange("p (db c) -> p db c", db=DB)
            for db in range(DB):
                nc.tensor.transpose(
                    pov[:, db, 0:P], osb[0:P, db * 128:(db + 1) * 128],
                    ident[0:P, 0:P],
                )
            nc.vector.tensor_copy(out=o_fm[:, :, bs:bs + S], in_=pov[:, :, 0:P])

        def wo_tile(t, wo_t, brow_t):
            ps = pmm.tile([128, 512], f32, tag="pmm", name="ps")
            for k in range(DB):
                nc.tensor.matmul(
                    ps[:, :],
                    o_fm[:, k, t * 128:(t + 1) * 128],
                    wo_t[:, k, :],
                    start=(k == 0), stop=False,
                )
            nc.tensor.matmul(
                ps[:, :], ones_row[0:1, :], brow_t[0:1, 0, :],
                start=False, stop=True,
            )
            nc.vector.tensor_add(x_t[:, t, :], x_t[:, t, :], ps[:, :])

        def w1_block(m, half, hfm2, w1a, w1b, b1_t, h_mlp):
            hc0 = half * 640
            w1_t = w1a if m < FB // 2 else w1b
            mm = m % (FB // 2)
            pmf = psc.tile([128, 1024], f32, tag="psc", name="pmf")
            for ci, (c0, cw) in enumerate(((hc0, 512), (hc0 + 512, 128))):
                for k in range(DB):
                    nc.tensor.matmul(
                        pmf[:, ci * 512:ci * 512 + cw],
                        w1_t[:, k, mm * 128:(mm + 1) * 128],
                        hfm2[:, k, c0:c0 + cw],
                        start=(k == 0), stop=(k == DB - 1),
                    )
            nc.scalar.activation(
                out=h_mlp[:, m, 0:640], in_=pmf[:, 0:640],
                func=AF.Gelu_apprx_sigmoid, bias=b1_t[:, m:m + 1],
            )

        def w2_tile(tt, half, h_mlp, w2a, w2b, brow_t):
            t = half * 5 + tt
            ps = pmm.tile([128, 512], f32, tag="pmm", name="ps")
            for k in range(FB):
                w2_t = w2a if k < FB // 2 else w2b
                nc.tensor.matmul(
                    ps[:, :],
                    h_mlp[:, k, tt * 128:(tt + 1) * 128],
                    w2_t[:, k % (FB // 2), :],
                    start=(k == 0), stop=False,
                )
            nc.tensor.matmul(
                ps[:, :], ones_row[0:1, :], brow_t[0:1, 1, :],
                start=False, stop=True,
            )
            nc.vector.tensor_add(x_t[:, t, :], x_t[:, t, :], ps[:, :])
            return t

        # ---------- layer 0 LN1 stats (x_t just loaded) ---------------------
        def alloc_mvbs():
            return [statp.tile([128, 4, 2], f32, tag=f"mvb{g}", name="mvb")
                    for g in range(3)]

        stats1 = alloc_mvbs()
        for t in range(NTILES):
            g, i = LN1_POS[t]
            emit_stat(stats1[g], i, t)

        # ---------- layer loop --------------------------------------------
        for li in range(NL):
            wq_t = wqkv.tile([128, DB, D], bf16, tag="wq", name="wq_t")
            nc.sync.dma_start(out=wq_t, in_=WQ[li].rearrange("(kb p) m -> p kb m", p=128))
            wk_t = wqkv.tile([128, DB, D], bf16, tag="wk", name="wk_t")
            nc.sync.dma_start(out=wk_t, in_=WK[li].rearrange("(kb p) m -> p kb m", p=128))
            wv_t = wqkv.tile([128, DB, D], bf16, tag="wv", name="wv_t")
            nc.sync.dma_start(out=wv_t, in_=WV[li].rearrange("(kb p) m -> p kb m", p=128))
            wo_t = wqkv.tile([128, DB, D], bf16, tag="wo", name="wo_t")
            nc.sync.dma_start(out=wo_t, in_=WO[li].rearrange("(kb p) m -> p kb m", p=128))
            w1a = wmlp.tile([128, DB, FF // 2], bf16, tag="w1a", name="w1a")
            nc.sync.dma_start(out=w1a, in_=W1[li, :, 0:FF // 2].rearrange("(kb p) m -> p kb m", p=128))
            w1b = wmlp.tile([128, DB, FF // 2], bf16, tag="w1b", name="w1b")
            nc.sync.dma_start(out=w1b, in_=W1[li, :, FF // 2:FF].rearrange("(kb p) m -> p kb m", p=128))
            w2a = wmlp.tile([128, FB // 2, D], bf16, tag="w2a", name="w2a")
            nc.sync.dma_start(out=w2a, in_=W2[li, 0:FF // 2, :].rearrange("(kb p) m -> p kb m", p=128))
            w2b = wmlp.tile([128, FB // 2, D], bf16, tag="w2b", name="w2b")
            nc.sync.dma_start(out=w2b, in_=W2[li, FF // 2:FF, :].rearrange("(kb p) m -> p kb m", p=128))
            bq_t = bp.tile([128, DB], f32, tag="bq", name="bq_t")
            nc.sync.dma_start(out=bq_t, in_=BQ[li])
            bk_t = bp.tile([128, DB], f32, tag="bk", name="bk_t")
            nc.sync.dma_start(out=bk_t, in_=BK[li])
            b1_t = bp.tile([128, FB], f32, tag="b1", name="b1_t")
            nc.sync.dma_start(out=b1_t, in_=B1[li])
            brow_t = browp.tile([1, 2, D], bf16, tag="brow", name="brow_t")
            nc.sync.dma_start(out=brow_t, in_=BROW[li])

            # ---- P1: LN1 finish + QKV + attention wavefront ----------------
            hfm = hfmp.tile([128, DB, NTILES * 128], bf16, tag="hfm", name="hfm")
            hfm2 = None
            # ln2 uses batches [0-3],[4,5],[6,7],[8,9] -> 4 mvb tiles
            ln2_mvbs = [statp.tile([128, 4, 2], f32, tag=f"l2mvb{g}", name="l2mvb")
                        for g in range(4)]
            LN2_POS = {}
            for g, batch in enumerate([[0, 1, 2, 3], [4, 5], [6, 7], [8, 9]]):
                for i, t in enumerate(batch):
                    LN2_POS[t] = (g, i)

            for (tiles, ci, seqs) in GROUPS:
                g = LN1_POS[tiles[0]][0]
                rs = emit_rsqrt(stats1[g], len(tiles))
                for i, t in enumerate(tiles):
                    ln_apply_tile(t, stats1[g], rs, i, hfm)
                qk_chunk(ci, hfm, wq_t, wk_t, bq_t, bk_t)
                for b in seqs:
                    v_proj(b, hfm, wv_t)
                    attention(b)
                    for t in WO_AT[b]:
                        wo_tile(t, wo_t, brow_t)
                        gg, ii = LN2_POS[t]
                        emit_stat(ln2_mvbs[gg], ii, t)
                    if b in LN2_BATCH_AT:
                        batch = LN2_BATCH_AT[b]
                        gg = LN2_POS[batch[0]][0]
                        rs2 = emit_rsqrt(ln2_mvbs[gg], len(batch))
                        if hfm2 is None:
                            hfm2 = hfmp.tile([128, DB, NTILES * 128], bf16,
                                             tag="hfm", name="hfm2")
                        for i, t in enumerate(batch):
                            ln_apply_tile(t, ln2_mvbs[gg], rs2, i, hfm2)

            # ---- P2: W1 + W2, interleaved with next-layer LN1 stats --------
            h_mlp0 = mlpp.tile([128, FB, 640], bf16, tag="hmlp", name="hmlp0")
            h_mlp1 = mlpp.tile([128, FB, 640], bf16, tag="hmlp", name="hmlp1")
            for m in range(FB):
                w1_block(m, 0, hfm2, w1a, w1b, b1_t, h_mlp0)

            stats_next = alloc_mvbs() if li < NL - 1 else None

            def after_w2(t):
                if li == NL - 1:
                    return
                for (b, p0) in PROMPT_ROWS.get(t, ()):
                    nc.sync.dma_start(
                        out=x_t[p0:p0 + NP, t, :], in_=PR[li, b]
                    )
                g, i = LN1_POS[t]
                emit_stat(stats_next[g], i, t)

            # W2 half0 interleaved with W1 half1
            for i in range(5):
                t = w2_tile(i, 0, h_mlp0, w2a, w2b, brow_t)
                after_w2(t)
                for m in (2 * i, 2 * i + 1):
                    w1_block(m, 1, hfm2, w1a, w1b, b1_t, h_mlp1)
            for m in range(10, FB):
                w1_block(m, 1, hfm2, w1a, w1b, b1_t, h_mlp1)
            for i in range(5):
                t = w2_tile(i, 1, h_mlp1, w2a, w2b, brow_t)
                after_w2(t)

            stats1 = stats_next

        nc.sync.dma_start(out=OUT[:, :, :], in_=x_t)
    return nc


_NC_CACHE = None
_LAST_IN_MAPS = None


def _get_nc():
    global _NC_CACHE
    if _NC_CACHE is None:
        _NC_CACHE = build_program()
    return _NC_CACHE


def kernel(text_tokens, attn_mask, g_prompt, s_prompt, token_emb, pos_emb,
           ln1_g, ln1_b, Wq, bq, Wk, bk, Wv, bv, Wo, bo,
           ln2_g, ln2_b, W1, b1, W2, b2, lnf_g, lnf_b):
    text_tokens = np.asarray(text_tokens)
    attn_mask = np.asarray(attn_mask)
    assert np.all(np.asarray(attn_mask) == 1), "kernel assumes all-ones attn_mask"
    fp = lambda a: np.asarray(a, dtype=np.float32)
    g_prompt, s_prompt = fp(g_prompt), fp(s_prompt)
    token_emb, pos_emb = fp(token_emb), fp(pos_emb)
    ln1_g, ln1_b, ln2_g, ln2_b = fp(ln1_g), fp(ln1_b), fp(ln2_g), fp(ln2_b)
    Wq, Wk, Wv, Wo, W1, W2 = fp(Wq), fp(Wk), fp(Wv), fp(Wo), fp(W1), fp(W2)
    bq, bk, bv, bo, b1, b2 = fp(bq), fp(bk), fp(bv), fp(bo), fp(b1), fp(b2)
    lnf_g, lnf_b = fp(lnf_g), fp(lnf_b)

    # ---- host-side input prep
    emb = token_emb[text_tokens]                                  # [B, T, D]
    x0 = np.concatenate([emb[:, :1], g_prompt[:, 0], emb[:, 1:]], axis=1)
    x0 = x0 + pos_emb[None, :S]                                   # [B, S, D]

    # fold LN gains/scale into weights
    wq_e = _bf(ln1_g[:, :, None] * Wq * SCALE)
    wk_e = _bf(ln1_g[:, :, None] * Wk)
    wv_e = _bf(ln1_g[:, :, None] * Wv)
    wo_e = _bf(Wo)
    w1_e = _bf(ln2_g[:, :, None] * W1)
    w2_e = _bf(W2)
    bq_e = (bq + np.einsum("ld,ldm->lm", ln1_b, Wq)) * SCALE      # [NL, D]
    bk_e = bk + np.einsum("ld,ldm->lm", ln1_b, Wk)
    bv_e = bv + np.einsum("ld,ldm->lm", ln1_b, Wv)
    b1_e = b1 + np.einsum("ld,ldm->lm", ln2_b, W1)
    # fold bv into bo: softmax rows sum to 1, so +bv commutes past attention
    bo_e = np.broadcast_to(bo, (NL, D)) + np.einsum("ld,ldm->lm", bv_e, Wo)
    bq_dev = _f32(bq_e.reshape(NL, DB, 128).transpose(0, 2, 1))
    bk_dev = _f32(bk_e.reshape(NL, DB, 128).transpose(0, 2, 1))
    b1_dev = _f32(b1_e.reshape(NL, FB, 128).transpose(0, 2, 1))
    brow = _bf(np.stack([bo_e, np.broadcast_to(b2, (NL, D))], axis=1))

    causalT = np.triu(np.ones((P, P), np.float32))                # [tk, tq]
    causal_m = np.zeros((P, H, 80), np.float32)
    causal_m[:, :, :P] = causalT[:, None, :]
    causal_rep = _bf(causal_m)
    ident = _bf(np.eye(128, dtype=np.float32))

    in_maps = []
    for c in range(NCORES):
        sl = slice(c * BSH, (c + 1) * BSH)
        flat = x0[sl].reshape(NT, D)
        flat = np.concatenate([flat, np.zeros((NTILES * 128 - NT, D), np.float32)])
        x0_dev = _f32(flat.reshape(NTILES, 128, D).transpose(1, 0, 2))
        prompts = np.stack(
            [g_prompt[sl, i] if i < DG else s_prompt[sl, i - (NL - DS)]
             for i in range(1, NL)]
        )
        in_maps.append({
            "x0": x0_dev, "prompts": _f32(prompts),
            "wq": wq_e, "wk": wk_e, "wv": wv_e, "wo": wo_e,
            "w1": w1_e, "w2": w2_e,
            "bq": bq_dev, "bk": bk_dev, "b1": b1_dev, "brow": brow,
            "causal": causal_rep, "ident": ident,
        })

    nc = _get_nc()
    global _LAST_IN_MAPS
    _LAST_IN_MAPS = in_maps
    res = run_bass_kernel_spmd(nc, in_maps, core_ids=list(range(NCORES)))

    # ---- host-side epilogue: final LN + EOT gather
    idx = np.argmax(text_tokens, axis=-1) + NP                    # [B]
    out = np.empty((B, D), np.float32)
    for c in range(NCORES):
        xr = res.results[c]["out"].transpose(1, 0, 2).reshape(NTILES * 128, D)
        for b in range(BSH):
            row = xr[(b * S) + idx[c * BSH + b]]
            m = row.mean()
            v = ((row - m) ** 2).mean()
            out[c * BSH + b] = (row - m) / np.sqrt(v + EPS) * lnf_g + lnf_b
    return out
